# revision 1
# baseline (speedup 1.0000x reference)
"""Fused pre-norm transformer block on 8 Trainium2 NeuronCores.

Problem: x[4,1024,1024]; LN -> qkv attention (16 heads) -> proj + residual;
LN -> fc1 -> gelu -> fc2 + residual.  dense_transformer, compute regime.

Sharding (no collectives): 2 cores per batch element, each owning 512 rows.
Each core redundantly computes K/V for its whole batch (key order is
softmax-invariant), so attention, MLP and both residuals are fully
core-local.  The host passes each core its batch's rows with the core's own
512 rows first; outputs are reassembled on host.

Numerics: LayerNorm and the attention-score chain run in float32r
(TF32-class, ~12-bit mantissa, full PE speed measured on-silicon); V /
attn / proj / MLP run in bf16 with fp32 PSUM accumulation.  LN rsqrt is
refined with two Newton steps (the ACT sqrt table alone is ~1e-3, which
softmax would amplify).

Attention layout: scores are computed transposed ([keys, rows]) so exp on
ScalarE writes bf16 attn^T directly in the layout the PV matmul consumes.
The softmax shift (exact per-row max, from a separate QK pass in [rows,
keys] orientation reduced on VectorE) is folded into the scores^T matmul
as a 65th contraction row (ones on the K side, -max on the Q side): on
this silicon consecutive matmuls with different contraction sizes cost
~3x (measured), so the shift must not be a separate rank-1 matmul and all
scores^T matmuls are a uniform K=65 f32r stream.  The softmax denominator
comes from an appended ones-column of V (self-consistent with the bf16
attn weights); its reciprocal is reshaped onto 128 partitions and its
broadcast bounced through a DRAM scratch tile (partition-stride-0 reads
are only legal from DRAM), keeping both off the PE and off the slow
1-lane VectorE path.
"""

import numpy as np
import ml_dtypes
from contextlib import ExitStack

import concourse.bass as bass
import concourse.tile as tile
from concourse import mybir
from concourse.bass_utils import run_bass_kernel_spmd
from concourse.vector_clock import ScopedClock, VectorClock
from concourse.masks import make_identity

F32 = mybir.dt.float32
F32R = mybir.dt.float32r
BF16 = mybir.dt.bfloat16
AF = mybir.ActivationFunctionType
OP = mybir.AluOpType
AX = mybir.AxisListType

B, N, C = 4, 1024, 1024
HEADS, DK = 16, 64
DFF = 4096
EPS = 1e-5
NB = 1024   # rows per core's batch (attention keys)
NO = 512    # rows owned per core
P = 128
CCH = C // P      # 8 chunks over C
MCH = NB // P     # 8 key-row chunks
OCH = NO // P     # 4 own-row chunks
FFCH = DFF // P   # 32
NHALF = NB // 512


class SplitDrainTileContext(tile.TileContext):
    """This walrus build rejects >2 sync waits on the tail SP drain
    ("Too many sync wait commands"); split the global-clock waits across
    single-wait drain instructions."""

    def _drain_and_barrier(self, tick_clock, wait_clock):
        nc = self.nc
        gc = tick_clock.global_clock
        n = len(gc)
        for i in range(n):
            if gc[i] > 0:
                vc = VectorClock([0] * n)
                vc.require_at_least(i, gc[i])
                d = nc.sync.drain()
                wait_clock.add_sem_waits(d.ins, ScopedClock({None: vc}))
        nc.sync.drain()
        nc.all_engine_barrier()
        popped = nc._tile_sem_poison_stack.pop()
        assert popped is self._sem_poison
        nc.clear_and_free_semaphores(list(self.sems.allocated().values()))
        nc.all_engine_barrier()


def legalize_waits(nc, cap=1):
    """Walrus here allows at most one sync wait per regular instruction.
    Hoist excess waits onto same-engine NoOps placed just before."""
    n = [0]

    def mknop(engine, wait):
        n[0] += 1
        nop = mybir.InstNoOp(name=f"I-waitfix-{n[0]}", ins=[], outs=[])
        nop.engine = engine
        nop.sync_info = mybir.SyncInfo(on_wait=[wait], on_update=[])
        return nop

    for f in nc.m.functions:
        for bb in f.blocks:
            out = []
            for inst in bb.instructions:
                w = list(inst.sync_info.on_wait or []) if inst.sync_info else []
                if len(w) > cap:
                    for extra in w[:-cap]:
                        out.append(mknop(inst.engine, extra))
                    inst.sync_info.on_wait = w[-cap:]
                out.append(inst)
            bb.instructions = out


def _rsqrt(nc, pool, var, eps, newton=2):
    """[128,1] fp32: 1/sqrt(var+eps); ACT-sqrt seed + Newton steps."""
    a = pool.tile([P, 1], F32, tag="rsq_a")
    nc.vector.tensor_scalar_add(a[:], var, eps)
    s0 = pool.tile([P, 1], F32, tag="rsq_s")
    nc.scalar.activation(s0[:], a[:], AF.Sqrt)
    r = pool.tile([P, 1], F32, tag="rsq_r")
    nc.vector.reciprocal(r[:], s0[:])
    t = pool.tile([P, 1], F32, tag="rsq_t")
    for _ in range(newton):
        nc.vector.tensor_mul(t[:], r[:], r[:])
        nc.vector.tensor_mul(t[:], t[:], a[:])
        nc.vector.tensor_scalar(t[:], t[:], -0.5, 1.5, op0=OP.mult, op1=OP.add)
        nc.vector.tensor_mul(r[:], r[:], t[:])
    return r


def _layer_norm_chunk(nc, pool, x_i, xn_i, newton=2):
    """LN of one [128, C] row-chunk: xn_i = (x - mean(x)) * rsqrt(var+eps)."""
    nsub = C // 512
    stats = pool.tile([P, nsub, 6], F32, tag="ln_stats")
    for s in range(nsub):
        nc.vector.bn_stats(stats[:, s, :], x_i[:, s * 512:(s + 1) * 512])
    mv = pool.tile([P, 2], F32, tag="ln_mv")
    nc.vector.bn_aggr(mv[:], stats[:])
    r = _rsqrt(nc, pool, mv[:, 1:2], EPS, newton=newton)
    nc.vector.tensor_scalar(
        xn_i[:], x_i[:], mv[:, 0:1], r[:], op0=OP.subtract, op1=OP.mult
    )


def build_program(legalize=True):
    nc = bass.Bass()

    x = nc.declare_dram_parameter("x", [NB, C], F32, isOutput=False)
    w_qk = nc.declare_dram_parameter("w_qk", [C, 2 * C], F32R, isOutput=False)
    w_v = nc.declare_dram_parameter("w_v", [C, C], F32R, isOutput=False)
    w_proj = nc.declare_dram_parameter("w_proj", [C, C], BF16, isOutput=False)
    w_fc1 = nc.declare_dram_parameter("w_fc1", [C, DFF], BF16, isOutput=False)
    w_fc2 = nc.declare_dram_parameter("w_fc2", [DFF, C], BF16, isOutput=False)
    b_qk = nc.declare_dram_parameter("b_qk", [2 * C], F32, isOutput=False)
    b_v = nc.declare_dram_parameter("b_v", [C], F32, isOutput=False)
    b_proj = nc.declare_dram_parameter("b_proj", [C], F32, isOutput=False)
    b_fc1 = nc.declare_dram_parameter("b_fc1", [DFF], F32, isOutput=False)
    b_fc2 = nc.declare_dram_parameter("b_fc2", [C], F32, isOutput=False)
    out = nc.declare_dram_parameter("out", [NO, C], F32, isOutput=True)

    with SplitDrainTileContext(nc) as tc:
        with ExitStack() as ctx:
            _build_body(
                nc, tc, ctx,
                x, w_qk, w_v, w_proj, w_fc1, w_fc2,
                b_qk, b_v, b_proj, b_fc1, b_fc2, out,
            )
    if legalize:
        legalize_waits(nc)
    return nc


def _build_body(nc, tc, ctx, x, w_qk, w_v, w_proj, w_fc1, w_fc2,
                b_qk, b_v, b_proj, b_fc1, b_fc2, out):
    perm = ctx.enter_context(tc.tile_pool(name="perm", bufs=1))
    small = ctx.enter_context(tc.tile_pool(name="small", bufs=3))

    # --- constants / biases (~37KB/partition in perm) ----------------------
    ident = perm.tile([P, P], F32)
    make_identity(nc, ident[:])

    bqk_sb = perm.tile([P, 2 * CCH], F32)
    nc.sync.dma_start(bqk_sb[:], b_qk.rearrange("(c p) -> p c", p=P))
    bproj_sb = perm.tile([P, CCH], F32)
    nc.sync.dma_start(bproj_sb[:], b_proj.rearrange("(c p) -> p c", p=P))
    bfc1_sb = perm.tile([P, FFCH], F32)
    nc.sync.dma_start(bfc1_sb[:], b_fc1.rearrange("(c p) -> p c", p=P))
    bfc2_sb = perm.tile([P, CCH], F32)
    nc.sync.dma_start(bfc2_sb[:], b_fc2.rearrange("(c p) -> p c", p=P))
    bv_bc = perm.tile([P, C], F32)
    nc.gpsimd.dma_start(
        bv_bc[:], bass.AP(tensor=b_v[:].tensor, offset=b_v[:].offset, ap=[[0, P], [1, C]])
    )
    ones_row = perm.tile([1, P], F32R)
    tmp1 = small.tile([1, P], F32)
    nc.vector.memset(tmp1[:], 1.0)
    nc.vector.tensor_copy(ones_row[:], tmp1[:])

    x_own = perm.tile([P, OCH, C], BF16)      # own rows (residual 1)
    x2 = perm.tile([P, OCH, C], BF16)         # post-attn residual stream

    with ExitStack() as kqv_scope:
        kqv = kqv_scope.enter_context(tc.tile_pool(name="kqv", bufs=1))
        kT = kqv.tile([P, CCH, NB], F32R)       # K^T head-pairs [128=2*dk, m]
        qT = kqv.tile([P, CCH, NO], F32R)       # Q^T head-pairs [128=2*dk, n_own]
        v_sb = kqv.tile([P, MCH, HEADS, DK + 1], BF16)  # V rows + ones col
        ctxT = kqv.tile([P, CCH, NO], BF16)     # (attn@V)^T, normalized

        def _emit_qk1_half(half, qk1pool, stgpool, trps):
            p0 = half * (CCH // 2)
            stg_h = stgpool.tile([P, CCH * OCH], F32, tag=f"stg{half}",
                                 name=f"stg{half}")
            for pp_ in range(p0, p0 + CCH // 2):
                for ncc in range(OCH):
                    base = ((pp_ - p0) * 2) * OCH + ncc
                    ps1a = qk1pool.tile([P, NB], F32, tag="ps1")
                    ps1b = qk1pool.tile([P, NB], F32, tag="ps1")
                    for mh in range(NHALF):
                        nc.tensor.matmul(
                            ps1a[:, mh * 512:(mh + 1) * 512],
                            qT[0:DK, pp_, ncc * P:(ncc + 1) * P],
                            kT[0:DK, pp_, mh * 512:(mh + 1) * 512],
                            start=True, stop=True)
                        nc.tensor.matmul(
                            ps1b[:, mh * 512:(mh + 1) * 512],
                            qT[DK:P, pp_, ncc * P:(ncc + 1) * P],
                            kT[DK:P, pp_, mh * 512:(mh + 1) * 512],
                            start=True, stop=True)
                    nc.vector.reduce_max(
                        stg_h[:, base:base + 1], ps1a[:], axis=AX.X, negate=True)
                    nc.vector.reduce_max(
                        stg_h[:, base + OCH:base + OCH + 1], ps1b[:],
                        axis=AX.X, negate=True)
            pstg = trps.tile([CCH * OCH, P], F32, tag="pstg", name=f"pstg{half}")
            nc.tensor.transpose(pstg[:], stg_h[:], ident[:])
            nc.vector.tensor_copy(stage2[half][:], pstg[0:CCH * OCH, :])
        stage2 = [kqv.tile([CCH * OCH, P], F32R, name=f"stage2_{i}")
                  for i in range(2)]

        with ExitStack() as ph_a:
            # ================ S1: load x, LN1, transpose ==================
            xnt_pool = ph_a.enter_context(tc.tile_pool(name="xnt", bufs=1))
            xnT = xnt_pool.tile([P, CCH, NB], F32R)   # LN1(x)^T  [c, n]
            ln = ph_a.enter_context(tc.tile_pool(name="ln", bufs=4))
            xoth = ph_a.enter_context(tc.tile_pool(name="xoth", bufs=2))
            tps = ph_a.enter_context(tc.tile_pool(name="tps", bufs=1, space="PSUM"))
            qkps = ph_a.enter_context(tc.tile_pool(name="qkps", bufs=2, space="PSUM"))
            pstgps = ph_a.enter_context(tc.tile_pool(name="pstgps", bufs=1, space="PSUM"))
            qk1ps = ph_a.enter_context(tc.tile_pool(name="qk1ps", bufs=2, space="PSUM"))
            sm = ph_a.enter_context(tc.tile_pool(name="sm", bufs=4))
            for i in range(MCH):
                x_i = xoth.tile([P, C], F32, tag="x_i")
                nc.sync.dma_start(x_i[:], x[i * P:(i + 1) * P, :])
                if i < OCH:
                    nc.scalar.copy(x_own[:, i, :], x_i[:])
                xn_i = xoth.tile([P, C], F32, tag="xn_i")
                _layer_norm_chunk(nc, ln, x_i, xn_i)
                for c in range(CCH):
                    pst = tps.tile([P, P], F32)
                    nc.tensor.transpose(pst[:], xn_i[:, c * P:(c + 1) * P], ident[:])
                    nc.vector.tensor_copy(xnT[:, c, i * P:(i + 1) * P], pst[:])

            # ================ S2: K^T, Q^T, V =============================
            wq = ph_a.enter_context(tc.tile_pool(name="wq", bufs=2 * CCH + 4))
            sm1 = ph_a.enter_context(tc.tile_pool(name="sm1", bufs=4))
            for p in range(CCH):
                # K^T pair p from w_qk cols C + p*128
                wk_t = []
                for c in range(CCH):
                    wt = wq.tile([P, P], F32R, tag="wqk_t")
                    nc.sync.dma_start(
                        wt[:], w_qk[c * P:(c + 1) * P, C + p * P:C + (p + 1) * P])
                    wk_t.append(wt)
                for nh in range(NHALF):
                    ps = qkps.tile([P, 512], F32, tag="qk_ps")
                    for c in range(CCH):
                        nc.tensor.matmul(
                            ps[:], wk_t[c][:], xnT[:, c, nh * 512:(nh + 1) * 512],
                            start=(c == 0), stop=(c == CCH - 1))
                    nc.scalar.activation(
                        kT[:, p, nh * 512:(nh + 1) * 512], ps[:], AF.Identity,
                        bias=bqk_sb[:, CCH + p:CCH + p + 1])
                # Q^T pair p
                wq_t = []
                for c in range(CCH):
                    wt = wq.tile([P, P], F32R, tag="wqk_t")
                    nc.sync.dma_start(wt[:], w_qk[c * P:(c + 1) * P, p * P:(p + 1) * P])
                    wq_t.append(wt)
                ps = qkps.tile([P, 512], F32, tag="qk_ps")
                for c in range(CCH):
                    nc.tensor.matmul(ps[:], wq_t[c][:], xnT[:, c, 0:NO],
                                     start=(c == 0), stop=(c == CCH - 1))
                nc.scalar.activation(qT[:, p, :], ps[:], AF.Identity,
                                     bias=bqk_sb[:, p:p + 1])
            _emit_qk1_half(0, qk1ps, sm1, pstgps)
            # V in natural [m, d] layout, bf16, ones column appended
            wv = ph_a.enter_context(tc.tile_pool(name="wv", bufs=1))
            for mc in range(MCH):
                nc.vector.memset(v_sb[:, mc, :, DK:DK + 1], 1.0)
            for dh in range(2):
                wvh = wv.tile([P, CCH, 512], F32R, tag="wvh")
                for c in range(CCH):
                    nc.sync.dma_start(
                        wvh[:, c, :], w_v[c * P:(c + 1) * P, dh * 512:(dh + 1) * 512])
                for mc in range(MCH):
                    ps = qkps.tile([P, 512], F32, tag="qk_ps")
                    for c in range(CCH):
                        nc.tensor.matmul(
                            ps[:], xnT[:, c, mc * P:(mc + 1) * P], wvh[:, c, :],
                            start=(c == 0), stop=(c == CCH - 1))
                    nc.vector.tensor_tensor(
                        out=v_sb[:, mc, dh * 8:(dh + 1) * 8, 0:DK],
                        in0=ps[:].rearrange("p (h d) -> p h d", d=DK),
                        in1=bv_bc[:, dh * 512:(dh + 1) * 512].rearrange(
                            "p (h d) -> p h d", d=DK),
                        op=OP.add,
                    )

        # ================ S3: attention ===================================
        # Per-head [65, ...] K/Q tiles (row 64 = softmax-shift augmentation:
        # ones on K, -rowmax on Q) are converted from the pair-packed tensors
        # just-in-time through a rotating pool, so all scores^T matmuls form
        # one uniform K=65 f32r stream across all 16 heads (K-size switches
        # between consecutive matmuls cost ~3x on this silicon).  The softmax
        # divide broadcast bounces through DRAM scratch (stride-0 partition
        # reads are only legal from DRAM), staying off the PE.
        with ExitStack() as ph_b:
            kq65 = ph_b.enter_context(tc.tile_pool(name="kq65", bufs=6))
            att = ph_b.enter_context(tc.tile_pool(name="att", bufs=4))
            sm = ph_b.enter_context(tc.tile_pool(name="sm", bufs=4))
            rbc = ph_b.enter_context(tc.tile_pool(name="rbc", bufs=3))
            drp = ph_b.enter_context(tc.tile_pool(name="drp", bufs=2, space="DRAM"))
            qk2ps = ph_b.enter_context(
                tc.tile_pool(name="qk2ps", bufs=3, space="PSUM"))
            ctxps = ph_b.enter_context(
                tc.tile_pool(name="ctxps", bufs=2, space="PSUM"))
            qk1psB = ph_b.enter_context(
                tc.tile_pool(name="qk1psB", bufs=1, space="PSUM"))
            ones_t = kq65.tile([1, NB], F32, tag="ones_t", bufs=1)
            nc.vector.memset(ones_t[:], 1.0)
            _emit_qk1_half(1, qk1psB, sm, qk1psB)

            pend = {}

            def _emit_norm(h):
                off2 = (h % 2) * DK
                ps3h, r_bch = pend.pop(h)
                nc.vector.tensor_tensor(
                    out=ctxT[off2:off2 + DK, h // 2, :], in0=ps3h[0:DK, :],
                    in1=r_bch[:], op=OP.mult)

            for h in range(HEADS):
                half, hh = h // CCH, h % CCH
                pp, off = h // 2, (h % 2) * DK
                kt65 = kq65.tile([DK + 1, NB], F32R, tag="kt65")
                qt65 = kq65.tile([DK + 1, NO], F32R, tag="qt65")
                nc.vector.tensor_copy(kt65[0:DK, :], kT[off:off + DK, pp, :])
                nc.vector.tensor_copy(qt65[0:DK, :], qT[off:off + DK, pp, :])
                nc.vector.tensor_copy(kt65[DK:DK + 1, :], ones_t[:])
                nc.sync.dma_start(qt65[DK:DK + 1, :],
                                  stage2[half][hh * OCH:(hh + 1) * OCH, :])
                attnT = att.tile([P, MCH, 512], BF16, tag="attnT")
                for mc in range(MCH):
                    ps2 = qk2ps.tile([P, 512], F32, tag="ps2")
                    nc.tensor.matmul(
                        ps2[:], kt65[:, mc * P:(mc + 1) * P], qt65[:],
                        start=True, stop=True)
                    nc.scalar.activation(attnT[:, mc, :], ps2[:], AF.Exp)
                ps3 = ctxps.tile([DK + 1, 512], F32, tag="ps3")
                for mc in range(MCH):
                    nc.tensor.matmul(ps3[:], v_sb[:, mc, h, :], attnT[:, mc, :],
                                     start=(mc == 0), stop=(mc == MCH - 1))
                s_row = sm.tile([1, 512], F32, tag="s_row")
                nc.vector.tensor_copy(s_row[:], ps3[DK:DK + 1, :])
                s_scr = drp.tile([1, 512], F32, tag="s_scr")
                nc.sync.dma_start(s_scr[:], s_row[:])
                s_sq = sm.tile([P, 4], F32, tag="s_sq")
                nc.sync.dma_start(
                    s_sq[:],
                    bass.AP(tensor=s_scr.tensor, offset=s_scr.offset,
                            ap=[[4, P], [1, 4]]))
                r_sq = sm.tile([P, 4], F32, tag="r_sq")
                nc.vector.reciprocal(r_sq[:], s_sq[:])
                r_scr = drp.tile([1, 512], F32, tag="r_scr")
                nc.sync.dma_start(
                    bass.AP(tensor=r_scr.tensor, offset=r_scr.offset,
                            ap=[[4, P], [1, 4]]), r_sq[:])
                r_bc = rbc.tile([DK, 512], F32, tag="r_bc")
                nc.sync.dma_start(
                    r_bc[:],
                    bass.AP(tensor=r_scr.tensor, offset=r_scr.offset,
                            ap=[[0, DK], [1, 512]]))
                pend[h] = (ps3, r_bc)
                if h >= 1:
                    _emit_norm(h - 1)
            _emit_norm(HEADS - 1)

        # ================ S4: proj + residual =============================
        with ExitStack() as ph_c:
            psb_pool = ph_c.enter_context(tc.tile_pool(name="psb", bufs=1))
            p_sb = psb_pool.tile([P, OCH, C], F32)
            wp = ph_c.enter_context(tc.tile_pool(name="wp", bufs=CCH + 2))
            pps = ph_c.enter_context(tc.tile_pool(name="pps", bufs=3, space="PSUM"))
            tps2 = ph_c.enter_context(tc.tile_pool(name="tps2", bufs=4, space="PSUM"))
            sc = ph_c.enter_context(tc.tile_pool(name="sc", bufs=3))
            for co in range(CCH):
                wp_t = []
                for c in range(CCH):
                    wt = wp.tile([P, P], BF16, tag="wp_t")
                    nc.sync.dma_start(
                        wt[:], w_proj[c * P:(c + 1) * P, co * P:(co + 1) * P])
                    wp_t.append(wt)
                ps = pps.tile([P, 512], F32)
                for c in range(CCH):
                    nc.tensor.matmul(ps[:], wp_t[c][:], ctxT[:, c, :],
                                     start=(c == 0), stop=(c == CCH - 1))
                pT_i = sc.tile([P, 512], F32, tag="pT_i")
                nc.vector.tensor_scalar_add(pT_i[:], ps[:], bproj_sb[:, co:co + 1])
                for ncc in range(OCH):
                    pst = tps2.tile([P, P], F32)
                    nc.tensor.transpose(
                        pst[:], pT_i[:, ncc * P:(ncc + 1) * P], ident[:])
                    nc.vector.tensor_copy(p_sb[:, ncc, co * P:(co + 1) * P], pst[:])
            for i in range(OCH):
                nc.vector.tensor_tensor(
                    out=x2[:, i, :], in0=x_own[:, i, :], in1=p_sb[:, i, :], op=OP.add)

    # ================ S4b: LN2 + transpose ================================
    with ExitStack() as mlp_scope:
        mlp = mlp_scope.enter_context(tc.tile_pool(name="mlp", bufs=1))
        x2nT = mlp.tile([P, CCH, NO], BF16)
        hT = mlp.tile([P, FFCH, NO], BF16)
        wf2 = mlp_scope.enter_context(tc.tile_pool(name="wf2", bufs=1))
        wf2_t = wf2.tile([P, FFCH, C], BF16)
        wf1_scope = ExitStack()
        wf1 = wf1_scope.enter_context(tc.tile_pool(name="wf1", bufs=1))
        wf1_t = wf1.tile([P, CCH, DFF], BF16)
        for c in range(CCH):
            nc.sync.dma_start(wf1_t[:, c, :], w_fc1[c * P:(c + 1) * P, :])
        for ff in range(FFCH):
            nc.sync.dma_start(wf2_t[:, ff, :], w_fc2[ff * P:(ff + 1) * P, :])
        with ExitStack() as ph_d:
            ln2 = ph_d.enter_context(tc.tile_pool(name="ln2", bufs=4))
            tps3 = ph_d.enter_context(tc.tile_pool(name="tps3", bufs=4, space="PSUM"))
            for i in range(OCH):
                x2n_i = ln2.tile([P, C], F32, tag="x2n_i")
                _layer_norm_chunk(nc, ln2, x2[:, i, :], x2n_i, newton=1)
                for c in range(CCH):
                    pst = tps3.tile([P, P], F32)
                    nc.tensor.transpose(
                        pst[:], x2n_i[:, c * P:(c + 1) * P], ident[:])
                    nc.vector.tensor_copy(x2nT[:, c, i * P:(i + 1) * P], pst[:])

        # ================ S5: fc1 + gelu ==================================
        with ExitStack() as ph_e:
            f1ps = ph_e.enter_context(tc.tile_pool(name="f1ps", bufs=4, space="PSUM"))
            for ff in range(FFCH):
                ps = f1ps.tile([P, 512], F32)
                for c in range(CCH):
                    nc.tensor.matmul(
                        ps[:], wf1_t[:, c, ff * P:(ff + 1) * P], x2nT[:, c, :],
                        start=(c == 0), stop=(c == CCH - 1))
                nc.scalar.activation(hT[:, ff, :], ps[:], AF.Gelu,
                                     bias=bfc1_sb[:, ff:ff + 1])

        wf1_scope.close()
        # ================ S6: fc2 + residual, store =======================
        with ExitStack() as ph_f:
            f2ps = ph_f.enter_context(tc.tile_pool(name="f2ps", bufs=1, space="PSUM"))
            tps4 = ph_f.enter_context(tc.tile_pool(name="tps4", bufs=4, space="PSUM"))
            sc2 = ph_f.enter_context(tc.tile_pool(name="sc2", bufs=3))
            o_pool = ph_f.enter_context(tc.tile_pool(name="o_pool", bufs=1))
            o_t = o_pool.tile([P, OCH, C], F32)
            for grp in range(2):
                psacc = [f2ps.tile([P, 512], F32, tag=f"f2acc{j}", name=f"f2acc{j}") for j in range(4)]
                for ff in range(FFCH):
                    for j in range(4):
                        co = grp * 4 + j
                        nc.tensor.matmul(
                            psacc[j][:], wf2_t[:, ff, co * P:(co + 1) * P], hT[:, ff, :],
                            start=(ff == 0), stop=(ff == FFCH - 1))
                for j in range(4):
                    co = grp * 4 + j
                    oT_i = sc2.tile([P, 512], F32, tag="oT_i")
                    nc.vector.tensor_scalar_add(
                        oT_i[:], psacc[j][:], bfc2_sb[:, co:co + 1])
                    for ncc in range(OCH):
                        pst = tps4.tile([P, P], F32)
                        nc.tensor.transpose(
                            pst[:], oT_i[:, ncc * P:(ncc + 1) * P], ident[:])
                        nc.vector.tensor_copy(
                            o_t[:, ncc, co * P:(co + 1) * P], pst[:])
            for i in range(OCH):
                fin = sc2.tile([P, C], F32, tag="fin")
                nc.vector.tensor_tensor(
                    out=fin[:], in0=x2[:, i, :], in1=o_t[:, i, :], op=OP.add)
                nc.sync.dma_start(out[i * P:(i + 1) * P, :], fin[:])


_NC_CACHE = [None]


def _get_nc():
    if _NC_CACHE[0] is None:
        _NC_CACHE[0] = build_program()
    return _NC_CACHE[0]


def _prepare_in_maps(inputs):
    f32 = lambda a: np.ascontiguousarray(np.asarray(a, dtype=np.float32))
    x = f32(inputs["x"])
    g = f32(inputs["norm_g"])
    bb = f32(inputs["norm_b"])
    w_qkv = f32(inputs["w_qkv"])
    b_qkv = f32(inputs["b_qkv"])
    w_proj = f32(inputs["w_proj"])
    b_proj = f32(inputs["b_proj"])
    w_fc1 = f32(inputs["w_fc1"])
    b_fc1 = f32(inputs["b_fc1"])
    w_fc2 = f32(inputs["w_fc2"])
    b_fc2 = f32(inputs["b_fc2"])

    # fold the LN affine into the consuming matmuls; fold the sqrt(dk)
    # score scale into w_q/b_q
    w_qkv_f = w_qkv * g[:, None]
    b_qkv_f = b_qkv + bb @ w_qkv
    scale = float(DK) ** 0.5
    w_q = w_qkv_f[:, 0:C] * scale
    b_q = b_qkv_f[0:C] * scale
    w_k = w_qkv_f[:, C:2 * C]
    b_k = b_qkv_f[C:2 * C]
    w_v = np.ascontiguousarray(w_qkv_f[:, 2 * C:3 * C])
    b_v = np.ascontiguousarray(b_qkv_f[2 * C:3 * C])
    w_fc1_f = w_fc1 * g[:, None]
    b_fc1_f = b_fc1 + bb @ w_fc1

    bf = lambda a: np.ascontiguousarray(a.astype(ml_dtypes.bfloat16))
    shared = {
        "w_qk": np.ascontiguousarray(np.concatenate([w_q, w_k], axis=1)),
        "w_v": w_v,
        "w_proj": bf(w_proj),
        "w_fc1": bf(w_fc1_f),
        "w_fc2": bf(w_fc2),
        "b_qk": np.ascontiguousarray(np.concatenate([b_q, b_k])),
        "b_v": b_v,
        "b_proj": b_proj,
        "b_fc1": np.ascontiguousarray(b_fc1_f),
        "b_fc2": b_fc2,
    }
    in_maps = []
    for core in range(8):
        b, half = core // 2, core % 2
        xb = x[b]
        x_core = np.ascontiguousarray(np.concatenate(
            [xb[half * NO:(half + 1) * NO], xb[(1 - half) * NO:(2 - half) * NO]],
            axis=0))
        in_maps.append({"x": x_core, **shared})
    return in_maps


def kernel(**inputs) -> np.ndarray:
    nc = _get_nc()
    in_maps = _prepare_in_maps(inputs)
    res = run_bass_kernel_spmd(nc, in_maps, list(range(8)))
    out = np.empty((B, N, C), dtype=np.float32)
    for core in range(8):
        b, half = core // 2, core % 2
        out[b, half * NO:(half + 1) * NO] = res.results[core]["out"]
    return out

